# revision 11
# baseline (speedup 1.0000x reference)
"""Trainium2 Bass kernel for nn_C_BatchNorm (complex batch-norm, training mode).

Problem: z [B=32, C=128, H=64, W=64, 2] fp32.  Per position n=(c,h,w):
  2x2 covariance over batch, closed-form inverse sqrt, whiten, gamma/beta.

Sharding: C split across 8 cores (16 channels each).  Per core the shard is
[32, 131072] fp32 (16 MiB in / 16 MiB out), processed as 32 tiles of
[128 partitions = 4 position-groups x 32 batch, 1024 = 512 positions x 2
interleaved components].

Algorithm per core (fp32 end-to-end, matmuls in f32r PE mode):
  Phase 1 (per tile): ACT square -> ZZ; DVE strided mult -> ZX=z0*z1;
    5 accumulating f32r matmuls with per-tile selector weights pack the raw
    moments S=sum(z), Q=sum(z^2), X=sum(z0 z1) for ALL tiles into PSUM rows
    32*(t//8) + 4*(t%8) + j  (j = position group).
  Phase 2 (once): closed-form 2x2 inverse-sqrt + gamma fold on the packed
    [128, 512]-per-row planes -> interleaved coefficient planes
    Pap=(A00,A10), Qap=(A01,A11), Rap=(R0,R1) where out = A.z + R.
  Phase 3 (per tile): K=32 indicator matmuls broadcast the tile's 4
    coefficient rows to all 128 partitions (replicated in PSUM); DVE
    multiplies with stride-0 "dup" views of z; GPSIMD adds the bias plane;
    result overwrites the resident z tile and is DMA'd out.
"""

import numpy as np

import concourse.bass as bass
import concourse.bacc as bacc
import concourse.tile as tile
from concourse import mybir
from concourse.bass_utils import run_bass_kernel_spmd

f32 = mybir.dt.float32
f32r = mybir.dt.float32r
AF = mybir.ActivationFunctionType
OP = mybir.AluOpType

# ---- problem geometry (hardcoded) ----
B, C, H, W = 32, 128, 64, 64
NCORES = 8
C_PER = C // NCORES                  # 16 channels per core
NPOS = C_PER * H * W                 # 65536 positions per core
M = NPOS * 2                         # 131072 fp32 per batch row per core
NT = 32                              # tiles per core
FP = 512                             # positions per group per tile
COLS = 2 * FP                        # 1024 fp32 per partition per tile
J = 4                                # position groups per tile (32 batch each)
NB = 512                             # matmul free-dim chunk (one PSUM bank)


def _host_constants():
    # selector weights for phase-1 stats packing: 8 variants [128, 32],
    # variant i: sel[p, w] = 1 iff w == 4*i + p//32
    sel8 = np.zeros((128, 8, 32), dtype=np.float32)
    for i in range(8):
        for p in range(128):
            sel8[p, i, 4 * i + p // 32] = 1.0
    sel8 = sel8.reshape(128, 8 * 32)
    # indicator for phase-3 broadcast: 8 variants [32, 128] tiled to 128 rows:
    # ind[p, 128*i + q] = 1 iff (p % 32) == 4*i + q//32
    ind = np.zeros((128, 8, 128), dtype=np.float32)
    for i in range(8):
        for p in range(128):
            for jj in range(4):
                if p % 32 == 4 * i + jj:
                    ind[p, i, 32 * jj:32 * (jj + 1)] = 1.0
    ind = ind.reshape(128, 8 * 128)
    return sel8, ind


def build_module():
    nc = bacc.Bacc("TRN2", target_bir_lowering=False, debug=False,
                   detect_race_conditions=False)
    z_d = nc.dram_tensor("z", [B, M], f32, kind="ExternalInput").ap()
    gamma_d = nc.dram_tensor("gamma", [2, 2], f32, kind="ExternalInput").ap()
    beta_d = nc.dram_tensor("beta", [2], f32, kind="ExternalInput").ap()
    sel8_d = nc.dram_tensor("sel8", [128, 8 * 32], f32, kind="ExternalInput").ap()
    ind_d = nc.dram_tensor("ind", [128, 8 * 128], f32, kind="ExternalInput").ap()
    out_d = nc.dram_tensor("out", [B, M], f32, kind="ExternalOutput").ap()

    # DRAM views ordered [tile, group, batch, col] (partition p = 32*j + b)
    z_r = z_d.rearrange("b (t j f) -> t j b f", t=NT, j=J, f=COLS)
    out_r = out_d.rearrange("b (t j f) -> t j b f", t=NT, j=J, f=COLS)

    irB = 1.0 / np.sqrt(np.float32(B))      # 1/sqrt(B)

    with tile.TileContext(nc) as tc:
        with (
            tc.tile_pool(name="consts", bufs=1) as consts,
            tc.tile_pool(name="zres", bufs=1) as zres,
            tc.tile_pool(name="stats", bufs=1) as stats,
            tc.tile_pool(name="ph2", bufs=1) as ph2,
            tc.tile_pool(name="work", bufs=2) as work,
            tc.tile_pool(name="workb", bufs=1) as workb,
        ):
            # ---------- constants ----------
            sel_sb = consts.tile([128, 8 * 32], f32)
            nc.sync.dma_start(out=sel_sb[:].bitcast(f32r),
                              in_=sel8_d.bitcast(f32r))
            ind_sb = consts.tile([128, 8 * 128], f32)
            nc.sync.dma_start(out=ind_sb[:].bitcast(f32r),
                              in_=ind_d.bitcast(f32r))

            gcols = consts.tile([128, 6], f32)   # g00 g01 g10 g11 b0 b1
            for k in range(4):
                nc.gpsimd.dma_start(
                    out=gcols[:, k:k + 1],
                    in_=bass.AP(tensor=gamma_d.tensor, offset=k,
                                ap=[[0, 128], [1, 1]]))
            for k in range(2):
                nc.gpsimd.dma_start(
                    out=gcols[:, 4 + k:5 + k],
                    in_=bass.AP(tensor=beta_d.tensor, offset=k,
                                ap=[[0, 128], [1, 1]]))
            g00c, g01c = gcols[:, 0:1], gcols[:, 1:2]
            g10c, g11c = gcols[:, 2:3], gcols[:, 3:4]
            b0c, b1c = gcols[:, 4:5], gcols[:, 5:6]

            # resident z for the whole core
            z_all = zres.tile([128, NT * COLS], f32)

            # ---------- phase 1: moments, packed into PSUM ----------
            Sint = stats.tile([128, COLS], f32)
            Qint = stats.tile([128, COLS], f32)
            Xp = stats.tile([128, FP], f32)
            with tc.tile_pool(name="psum1", bufs=1, space="PSUM") as psum1:
                # f32r matmuls may only write PSUM partition base 0, so each
                # 8-tile block (g) accumulates its 32 stats rows at
                # partitions 0-31, which are then staged + DMA-repacked to
                # partitions 32g of the Sint/Qint/Xp tensors.
                ps_S = psum1.tile([32, COLS], f32)
                ps_Q = psum1.tile([32, COLS], f32)
                ps_X = psum1.tile([32, FP], f32)
                for t in range(NT):
                    g, i = divmod(t, 8)
                    zv = z_all[:, t * COLS:(t + 1) * COLS]
                    nc.sync.dma_start(out=zv.bitcast(f32r), in_=z_r[t].bitcast(f32r))
                    zz = work.tile([128, COLS], f32, tag="zz")
                    nc.scalar.square(zz[:].bitcast(f32r), zv)
                    zx = work.tile([128, FP], f32, tag="zx")
                    z_ev = bass.AP(tensor=z_all.tensor, offset=t * COLS,
                                   ap=[list(z_all.ap[0]), [2, FP]])
                    z_od = bass.AP(tensor=z_all.tensor, offset=t * COLS + 1,
                                   ap=[list(z_all.ap[0]), [2, FP]])
                    nc.gpsimd.tensor_tensor(zx[:].bitcast(f32r), z_ev, z_od, OP.mult)

                    lhs = sel_sb[:, 32 * i:32 * (i + 1)].bitcast(f32r)
                    # first tile of each block zeroes the bank region
                    st = i == 0
                    sp = i == 7
                    for h in range(2):
                        cs = slice(h * NB, (h + 1) * NB)
                        nc.tensor.matmul(ps_S[:, cs], lhs,
                                         zv[:, cs].bitcast(f32r),
                                         start=st, stop=sp,
                                         tile_position=(0, 0),
                                         skip_group_check=True)
                        nc.tensor.matmul(ps_Q[:, cs], lhs,
                                         zz[:, cs].bitcast(f32r),
                                         start=st, stop=sp,
                                         tile_position=(0, 0),
                                         skip_group_check=True)
                    nc.tensor.matmul(ps_X[:, :], lhs,
                                     zx[:].bitcast(f32r),
                                     start=st, stop=sp,
                                     tile_position=(0, 0),
                                     skip_group_check=True)

                    if i == 7:
                        # stage this block's stats and repack to rows 32g
                        stg = workb.tile([32, 2 * COLS + FP], f32, tag="stg")
                        nc.scalar.copy(stg[:, 0:COLS], ps_S)
                        nc.scalar.copy(stg[:, COLS:2 * COLS], ps_Q)
                        nc.scalar.copy(stg[:, 2 * COLS:], ps_X)
                        rows = slice(32 * g, 32 * (g + 1))
                        nc.sync.dma_start(out=Sint[rows, :],
                                          in_=stg[:, 0:COLS])
                        nc.sync.dma_start(out=Qint[rows, :],
                                          in_=stg[:, COLS:2 * COLS])
                        nc.sync.dma_start(out=Xp[rows, :],
                                          in_=stg[:, 2 * COLS:])

            # ---------- phase 2: closed-form 2x2 inverse sqrt + gamma ----------
            # views
            def ev(t_, n=FP):
                return bass.AP(tensor=t_.tensor, offset=t_.offset,
                               ap=[list(t_.ap[0]), [2, n]])

            def od(t_, n=FP):
                return bass.AP(tensor=t_.tensor, offset=t_.offset + 1,
                               ap=[list(t_.ap[0]), [2, n]])

            def dup(t_, n=FP):    # [n] -> [n,2] with stride-0 inner
                return bass.AP(tensor=t_.tensor, offset=t_.offset,
                               ap=[list(t_.ap[0]), [1, n], [0, 2]])

            def pair(t_, n=FP):   # [2n] viewed as [n,2]
                return bass.AP(tensor=t_.tensor, offset=t_.offset,
                               ap=[list(t_.ap[0]), [2, n], [1, 2]])

            Pint = ph2.tile([128, COLS], f32)
            P01 = ph2.tile([128, FP], f32)
            d1 = ph2.tile([128, FP], f32)
            s_ = ph2.tile([128, FP], f32)
            u_ = ph2.tile([128, FP], f32)
            r_ = ph2.tile([128, FP], f32)
            # aliases onto dead scratch (lifetimes are disjoint):
            q2 = P01     # P01 dead once sigma01 subtract is done
            u2 = d1      # d1 dead once s_ = sqrt(d1)
            tq = u_      # u_ dead once u2 = 2s + u_
            rsc = P01    # q2 dead once det -= q2
            T_ = s_      # s_ dead once e_int += s dup
            c0 = d1      # u2 dead once tq = sqrt((B-1) u2)
            c1 = r_      # r_ dead once w01 *= r
            Pap = Pint   # Pint dead once sigma~ = Q - P is done
            Qap = ph2.tile([128, COLS], f32)
            Rap = ph2.tile([128, COLS], f32)

            # Pint = (S/sqrt(B))^2 per component (interleaved)
            nc.scalar.activation(Pint[:].bitcast(f32r), Sint, AF.Square,
                                 scale=float(irB))
            # P01 = (S0/B)*S1
            nc.vector.scalar_tensor_tensor(P01, ev(Sint), float(irB * irB),
                                           od(Sint), OP.mult, OP.mult)
            # sigma~ (in-place into Qint / Xp)
            nc.vector.tensor_tensor(Qint, Qint, Pint, OP.subtract)
            nc.vector.tensor_tensor(Xp, Xp, P01, OP.subtract)
            # det
            nc.vector.tensor_tensor(d1, ev(Qint), od(Qint), OP.mult)
            nc.scalar.square(q2, Xp)
            nc.vector.tensor_tensor(d1, d1, q2, OP.subtract)
            nc.scalar.activation(s_, d1, AF.Sqrt)
            # u = trace~ + 2 s~
            nc.vector.tensor_tensor(u_, ev(Qint), od(Qint), OP.add)
            nc.vector.scalar_tensor_tensor(u2, s_, 2.0, u_, OP.mult, OP.add)
            # tq = sqrt((B-1) u) = (B-1) t ;  r = 1/tq
            nc.scalar.activation(tq, u2, AF.Sqrt, scale=float(B - 1))
            nc.vector.reciprocal_approx_accurate(r_, tq, rsc)
            # e = sigma~ + s~ I (dup) ; W = e*r ; w01 = sigma01 * r
            nc.vector.tensor_tensor(pair(Qint), pair(Qint), dup(s_), OP.add)
            nc.vector.tensor_tensor(pair(Qint), pair(Qint), dup(r_), OP.mult)
            nc.vector.tensor_tensor(Xp, Xp, r_, OP.mult)
            w00, w11, w01 = ev(Qint), od(Qint), Xp

            # A coefficients -> interleaved apply planes
            # Pap = (A00, A10): A00 = g00 w00 + g01 w01 ; A10 = g10 w00 + g11 w01
            nc.vector.tensor_scalar(T_, w00, g00c, None, OP.mult)
            nc.vector.scalar_tensor_tensor(ev(Pap).bitcast(f32r), w01, g01c, T_, OP.mult, OP.add)
            nc.vector.tensor_scalar(T_, w00, g10c, None, OP.mult)
            nc.vector.scalar_tensor_tensor(od(Pap).bitcast(f32r), w01, g11c, T_, OP.mult, OP.add)
            # Qap = (A01, A11): A01 = g00 w01 + g01 w11 ; A11 = g10 w01 + g11 w11
            nc.vector.tensor_scalar(T_, w11, g01c, None, OP.mult)
            nc.vector.scalar_tensor_tensor(ev(Qap).bitcast(f32r), w01, g00c, T_, OP.mult, OP.add)
            nc.vector.tensor_scalar(T_, w11, g11c, None, OP.mult)
            nc.vector.scalar_tensor_tensor(od(Qap).bitcast(f32r), w01, g10c, T_, OP.mult, OP.add)
            # Rap = (R0, R1): R0 = b0 - (A00 S0 + A01 S1)/B
            nc.vector.tensor_tensor(c0, ev(Pap), ev(Sint), OP.mult)
            nc.vector.tensor_tensor(c1, ev(Qap), od(Sint), OP.mult)
            nc.vector.tensor_tensor(c0, c0, c1, OP.add)
            nc.vector.tensor_scalar(ev(Rap).bitcast(f32r), c0, float(-1.0 / B),
                                    b0c, OP.mult, OP.add)
            nc.vector.tensor_tensor(c0, od(Pap), ev(Sint), OP.mult)
            nc.vector.tensor_tensor(c1, od(Qap), od(Sint), OP.mult)
            nc.vector.tensor_tensor(c0, c0, c1, OP.add)
            nc.vector.tensor_scalar(od(Rap).bitcast(f32r), c0, float(-1.0 / B),
                                    b1c, OP.mult, OP.add)

            # ---------- phase 3: broadcast + apply ----------
            with tc.tile_pool(name="psum3", bufs=1, space="PSUM") as psum3:
                ps_P = psum3.tile([128, COLS], f32)
                ps_Qb = psum3.tile([128, COLS], f32)
                ps_R = psum3.tile([128, COLS], f32)
                for t in range(NT):
                    g, i = divmod(t, 8)
                    rows = slice(32 * g, 32 * (g + 1))
                    lhs_b = ind_sb[rows, 128 * i:128 * (i + 1)].bitcast(f32r)
                    for h in range(2):
                        cs = slice(h * NB, (h + 1) * NB)
                        nc.tensor.matmul(ps_P[:, cs], lhs_b,
                                         Pap[rows, cs].bitcast(f32r),
                                         start=True, stop=True,
                                         tile_position=(32 * g, 0),
                                         skip_group_check=True)
                        nc.tensor.matmul(ps_Qb[:, cs], lhs_b,
                                         Qap[rows, cs].bitcast(f32r),
                                         start=True, stop=True,
                                         tile_position=(32 * g, 0),
                                         skip_group_check=True)
                        nc.tensor.matmul(ps_R[:, cs], lhs_b,
                                         Rap[rows, cs].bitcast(f32r),
                                         start=True, stop=True,
                                         tile_position=(32 * g, 0),
                                         skip_group_check=True)
                    zoff = t * COLS
                    zdup_ev = bass.AP(tensor=z_all.tensor, offset=zoff,
                                      ap=[list(z_all.ap[0]), [2, FP], [0, 2]])
                    zdup_od = bass.AP(tensor=z_all.tensor, offset=zoff + 1,
                                      ap=[list(z_all.ap[0]), [2, FP], [0, 2]])
                    t1 = work.tile([128, COLS], f32, tag="t1")
                    t2 = work.tile([128, COLS], f32, tag="zz")
                    nc.vector.tensor_tensor(pair(t1), pair(ps_P), zdup_ev,
                                            OP.mult)
                    nc.vector.tensor_tensor(pair(t2), pair(ps_Qb), zdup_od,
                                            OP.mult)
                    nc.vector.tensor_tensor(t1, t1, t2, OP.add)
                    rsb = workb.tile([128, COLS], f32, tag="stg")
                    nc.scalar.copy(rsb, ps_R)
                    ob = work.tile([128, COLS], f32, tag="outb")
                    nc.gpsimd.tensor_tensor(ob, t1, rsb, OP.add)
                    nc.scalar.dma_start(out=out_r[t], in_=ob)

    nc.compile()
    return nc


_NC = None


def _get_module():
    global _NC
    if _NC is None:
        _NC = build_module()
    return _NC


def kernel(z, gamma, beta):
    z = np.ascontiguousarray(z, dtype=np.float32)
    gamma = np.ascontiguousarray(gamma, dtype=np.float32)
    beta = np.ascontiguousarray(beta, dtype=np.float32)
    zr = z.reshape(B, C, H * W * 2)
    sel8, ind = _host_constants()
    in_maps = []
    for c in range(NCORES):
        shard = np.ascontiguousarray(
            zr[:, c * C_PER:(c + 1) * C_PER].reshape(B, M))
        in_maps.append({"z": shard, "gamma": gamma, "beta": beta,
                        "sel8": sel8, "ind": ind})
    nc = _get_module()
    res = run_bass_kernel_spmd(nc, in_maps, core_ids=list(range(NCORES)))
    out = np.empty((B, C, H * W * 2), dtype=np.float32)
    for c in range(NCORES):
        out[:, c * C_PER:(c + 1) * C_PER] = res.results[c]["out"].reshape(
            B, C_PER, H * W * 2)
    return out.reshape(B, C, H, W, 2)


def _build_memcpy_module():
    """Baseline: per-core DMA z -> out through SBUF (same traffic as kernel)."""
    nc = bacc.Bacc("TRN2", target_bir_lowering=False, debug=False,
                   detect_race_conditions=False)
    z_d = nc.dram_tensor("z", [B, M], f32, kind="ExternalInput").ap()
    out_d = nc.dram_tensor("out", [B, M], f32, kind="ExternalOutput").ap()
    z_r = z_d.rearrange("b (t j f) -> t j b f", t=NT, j=J, f=COLS)
    out_r = out_d.rearrange("b (t j f) -> t j b f", t=NT, j=J, f=COLS)
    with tile.TileContext(nc) as tc:
        with tc.tile_pool(name="buf", bufs=4) as buf:
            for t in range(NT):
                x = buf.tile([128, COLS], f32, tag="x")
                nc.sync.dma_start(out=x[:], in_=z_r[t])
                nc.scalar.dma_start(out=out_r[t], in_=x[:])
    nc.compile()
    return nc


def bench_module(nc, in_maps, iters=12):
    """Amortized async timing of an SPMD bass module via the PJRT path.

    Returns (per_iter_ns, results_of_last_iter). Inputs are uploaded once;
    `iters` sets of donated output buffers are pre-placed on device; the
    executions queue asynchronously so dispatch overhead overlaps device
    work.
    """
    import time as _time

    import jax
    import jax.numpy as jnp
    from jax.sharding import Mesh, PartitionSpec
    from jax.experimental.shard_map import shard_map
    from concourse import bass2jax
    from concourse.bass2jax import _bass_exec_p, install_neuronx_cc_hook
    from concourse import mybir as _mb

    install_neuronx_cc_hook()
    n_cores = len(in_maps)
    partition_name = (nc.partition_id_tensor.name
                      if nc.partition_id_tensor else None)
    in_names, out_names, out_avals, zero_outs = [], [], [], []
    for alloc in nc.m.functions[0].allocations:
        if not isinstance(alloc, _mb.MemoryLocationSet):
            continue
        name = alloc.memorylocations[0].name
        if alloc.kind == "ExternalInput":
            if name != partition_name:
                in_names.append(name)
        elif alloc.kind == "ExternalOutput":
            shape = tuple(alloc.tensor_shape)
            dtype = _mb.dt.np(alloc.dtype)
            out_names.append(name)
            out_avals.append(jax.core.ShapedArray(shape, dtype))
            zero_outs.append(np.zeros(shape, dtype))
    n_params = len(in_names)
    n_outs = len(out_avals)
    all_in_names = in_names + out_names
    if partition_name is not None:
        all_in_names.append(partition_name)

    def _body(*args):
        operands = list(args)
        if partition_name is not None:
            operands.append(bass2jax.partition_id_tensor())
        outs = _bass_exec_p.bind(
            *operands,
            out_avals=tuple(out_avals),
            in_names=tuple(all_in_names),
            out_names=tuple(out_names),
            lowering_input_output_aliases=(),
            sim_require_finite=True,
            sim_require_nnan=True,
            nc=nc,
        )
        return tuple(outs)

    devices = jax.devices()[:n_cores]
    mesh = Mesh(np.asarray(devices), ("core",))
    donate = tuple(range(n_params, n_params + n_outs))
    sharded = jax.jit(
        shard_map(_body, mesh=mesh,
                  in_specs=(PartitionSpec("core"),) * (n_params + n_outs),
                  out_specs=(PartitionSpec("core"),) * n_outs,
                  check_rep=False),
        donate_argnums=donate, keep_unused=True,
    )
    from jax.sharding import NamedSharding
    shard0 = NamedSharding(mesh, PartitionSpec("core"))
    concat_in = [
        jax.device_put(
            np.concatenate([np.asarray(m[name]) for m in in_maps], axis=0),
            shard0)
        for name in in_names
    ]
    zero_sets = []
    for _ in range(iters + 1):
        zero_sets.append([
            jax.device_put(
                np.zeros((n_cores * z.shape[0], *z.shape[1:]), z.dtype),
                shard0)
            for z in zero_outs
        ])
    # warmup (compiles)
    outs = sharded(*concat_in, *zero_sets[0])
    jax.block_until_ready(outs)
    t0 = _time.perf_counter()
    last = None
    for k in range(iters):
        last = sharded(*concat_in, *zero_sets[k + 1])
    jax.block_until_ready(last)
    dt = (_time.perf_counter() - t0) / iters
    results = [
        {name: np.asarray(last[i]).reshape(n_cores, *out_avals[i].shape)[c]
         for i, name in enumerate(out_names)}
        for c in range(n_cores)
    ]
    return dt * 1e9, results


def bench(z, gamma, beta, iters=12, with_memcpy=True):
    z = np.ascontiguousarray(z, dtype=np.float32)
    zr = z.reshape(B, C, H * W * 2)
    sel8, ind = _host_constants()
    in_maps = []
    for c in range(NCORES):
        shard = np.ascontiguousarray(
            zr[:, c * C_PER:(c + 1) * C_PER].reshape(B, M))
        in_maps.append({"z": shard,
                        "gamma": np.ascontiguousarray(gamma, np.float32),
                        "beta": np.ascontiguousarray(beta, np.float32),
                        "sel8": sel8, "ind": ind})
    ns, results = bench_module(_get_module(), in_maps, iters=iters)
    out = np.empty((B, C, H * W * 2), dtype=np.float32)
    for c in range(NCORES):
        out[:, c * C_PER:(c + 1) * C_PER] = results[c]["out"].reshape(
            B, C_PER, H * W * 2)
    memcpy_ns = None
    if with_memcpy:
        mc = _build_memcpy_module()
        mc_maps = [{"z": m["z"]} for m in in_maps]
        memcpy_ns, _ = bench_module(mc, mc_maps, iters=iters)
    return out.reshape(B, C, H, W, 2), ns, memcpy_ns


def run_traced(z, gamma, beta):
    """Like kernel() but with NTFF tracing; returns (output, exec_time_ns)."""
    z = np.ascontiguousarray(z, dtype=np.float32)
    zr = z.reshape(B, C, H * W * 2)
    sel8, ind = _host_constants()
    in_maps = []
    for c in range(NCORES):
        shard = np.ascontiguousarray(
            zr[:, c * C_PER:(c + 1) * C_PER].reshape(B, M))
        in_maps.append({"z": shard,
                        "gamma": np.ascontiguousarray(gamma, np.float32),
                        "beta": np.ascontiguousarray(beta, np.float32),
                        "sel8": sel8, "ind": ind})
    nc = _get_module()
    res = run_bass_kernel_spmd(nc, in_maps, core_ids=list(range(NCORES)),
                               trace=True)
    out = np.empty((B, C, H * W * 2), dtype=np.float32)
    for c in range(NCORES):
        out[:, c * C_PER:(c + 1) * C_PER] = res.results[c]["out"].reshape(
            B, C_PER, H * W * 2)
    return out.reshape(B, C, H, W, 2), res.exec_time_ns, res
